# revision 2
# baseline (speedup 1.0000x reference)
"""Trainium2 Bass kernel for the SG-visibility sampling network (v2).

Math notes (exploited structure):
  - U,V are orthogonal to the unit lobe axis l, so dot(sample_dir, l) == cos(r_phi)
    exactly.  Hence the SG weight w = exp(sharp*(cos_phi-1)) is a per-lobe
    constant and sum_s(vis*w)/(sum_s w + TINY) = scale_l * sum_s vis with
    scale_l = w/(S*w + TINY), precomputed on host.
  - pre-activation of the hidden layer decomposes as
        pre_h[n,l,s,h] = P_n[h] - C_l[h] - ct[n,l,s]*A_l[h] - st[n,l,s]*B_l[h]
    with P_n = p_n @ W1[:3] + b1,  A_l = sp_l*(U_l@Wd),  B_l = sp_l*(V_l@Wd),
    C_l = cp_l*(l_l@Wd),  Wd = root_rot @ W1[3:].
  - hemisphere mask: cos_term = ct*a_nl + st*b_nl + c_nl with
    a = normals@(sp*U)_l, b = normals@(sp*V)_l, c = normals@(cp*l)_l.
  - sigmoid(z) = 0.5*tanh(z/2) + 0.5, so vis*msk = 0.5*(tanh+1)*msk and the
    weighted sum runs as ONE accumulating matmul per (chunk, half) with
    scale_l/2 stationary.  Using Tanh instead of Sigmoid keeps every
    activation (Sin/Tanh/Relu/Copy) in ONE ACT table set (silu_and_others)
    -- zero LoadActFuncSet switches in steady state (the table-map cache is
    narrowed in _build_program so the greedy placement pass must pick it).

Device schedule (per core, data-parallel over N):
  - mask path in full fp32 (sign-exact): per-lobe a/b/c dots as 6 tiny fp32
    matmuls (TINY folded in via an ones-row so the compare is a single
    tensor_tensor is_gt), s-duplicated per chunk by one broadcast DMA each;
    mask trig = 2 fp32 Sin per chunk; q1/q2/q3/cmp + (tanh+1)*msk on DVE,
    issued AFTER each half's relu-drains so the DVE FIFO never blocks them.
  - hidden path in bf16: theta is HOST-PREBAKED into duplicated block layout
    (ct-rows | st-rows | point-rows per {7,7,2} sub-chunk) so one in-place Sin
    per sub-chunk tile produces the moving operand directly; per-lobe hidden
    matmul (bf16, K=116/36) runs LOOK=3 lobes ahead of the z matmuls (relu
    drain latency ~0.9us vs 426ns PE work per lobe), relu-drains alternate
    ACT/DVE (Pool cannot touch PSUM and walrus rejects ALU ops on Pool),
    K=128 bf16 matmul against block-diag W2 -> z in PSUM (double-buffered),
    one tanh per half, and a deferred scale matmul into the [L, n] output
    PSUM (issued mid-way through the NEXT half's z stream to dodge PE
    FIFO head-of-line blocking).
"""

import numpy as np

N, L, S, H = 8192, 128, 8, 16
NCORES = 8
NC = N // NCORES          # rays per core
LPC = 16                  # lobes per chunk
CHUNKS = L // LPC
TINY = 1e-6
HF = NC // 2              # moving-operand free-dim limit (512)

# inp (f32) row layout
R_RT = 0                  # rows 0..1023: r_theta in [l*S+s, n] layout
R_NRM = L * S             # rows 1024..1027: normals^T (3) + ones row
R_WABC = R_NRM + 4        # rows 1028..1031: wabc [4, 3*L] in cols 0..383
R_CB = R_WABC + 4         # rows 1032..1159: cb [128, 8] in cols 0..7
INP_ROWS = R_CB + 128

# inpb (bf16): per chunk 268 rows = [116 | 116 | 36] sub-chunk tiles
BLK_ROWS = 268
SUBS = ((0, 7), (7, 7), (14, 2))   # (first lobe-in-chunk, n lobes) per sub-tile

# wcb (bf16) col layout
C_WCST = 0                          # [128, L*128]
C_WSIG = L * 128                    # [128, 512]
C_WSUM = C_WSIG + 512               # [128, CHUNKS*128]
WCB_COLS = C_WSUM + CHUNKS * 128

_PROG = None


def _build_program():
    import concourse.bass as bass
    import concourse.bacc as bacc
    import concourse.mybir as mybir
    import concourse.tile as tile

    f32 = mybir.dt.float32
    bf16 = mybir.dt.bfloat16
    AF = mybir.ActivationFunctionType
    ALU = mybir.AluOpType
    PI4 = float(np.pi / 4.0)

    nc = bacc.Bacc("TRN2", target_bir_lowering=False, debug=False,
                   num_devices=NCORES)

    # The act-table-load pass greedily picks the FIRST table set containing
    # each activation's func, which alternates trig_and_small <-> exp_and_others
    # for Sin/Tanh (38 reloads, ~50us serialized on ACT).  silu_and_others
    # genuinely contains Sin+Tanh+Relu together; constrain the (cached) table
    # map so the pass can only pick it for Sin/Tanh.  Set ids are positional,
    # so entries are mutated in place -- never reordered.
    from concourse.hw_specs import get_activation_tables
    tabs = get_activation_tables(nc.m.arch)
    assert {AF.Sin, AF.Tanh, AF.Relu} <= tabs["silu_and_others"]
    for name, funcs in tabs.items():
        if name != "silu_and_others":
            funcs.discard(AF.Sin)
            funcs.discard(AF.Tanh)

    inp = nc.declare_dram_parameter("inp", [INP_ROWS, NC], f32, isOutput=False)
    inpb = nc.declare_dram_parameter("inpb", [CHUNKS * BLK_ROWS, NC], bf16,
                                     isOutput=False)
    wcb = nc.declare_dram_parameter("wcb", [128, WCB_COLS], bf16, isOutput=False)
    out = nc.declare_dram_parameter("out", [L, NC], f32, isOutput=True)

    # relu-drain engine per lobe-in-chunk (Pool/gpsimd cannot read PSUM, so
    # drains alternate ACT / DVE; Pool owns the SBUF-only mask chain)
    DRAIN = "ADADADADADADADAD"
    assert len(DRAIN) == LPC

    with tile.TileContext(nc) as tc:
        with (
            tc.tile_pool(name="const", bufs=1) as cpool,
            tc.tile_pool(name="io", bufs=3) as io,
            tc.tile_pool(name="wstage", bufs=3) as wstage,
            tc.tile_pool(name="trig", bufs=2) as trig,
            tc.tile_pool(name="abc", bufs=2) as abcp,
            tc.tile_pool(name="work", bufs=3) as work,
            tc.tile_pool(name="hrp", bufs=5) as hrp,
            tc.tile_pool(name="ps", bufs=4, space=bass.MemorySpace.PSUM) as ps,
            tc.tile_pool(name="zps", bufs=2, space=bass.MemorySpace.PSUM) as zps,
            tc.tile_pool(name="ops", bufs=1, space=bass.MemorySpace.PSUM) as opsp,
        ):
            nrm4_t = cpool.tile([4, NC], f32)
            nc.sync.dma_start(nrm4_t[:], inp[R_NRM:R_NRM + 4, :])
            wabc_t = cpool.tile([4, 3 * L], f32)
            nc.sync.dma_start(wabc_t[:], inp[R_WABC:R_WABC + 4, 0:3 * L])
            cb_t = cpool.tile([128, 8], f32)
            nc.sync.dma_start(cb_t[:], inp[R_CB:R_CB + 128, 0:8])
            wsig_t = cpool.tile([128, 512], bf16)
            nc.sync.dma_start(wsig_t[:], wcb[:, C_WSIG:C_WSIG + 512])
            wsum_t = cpool.tile([128, CHUNKS * 128], bf16)
            nc.sync.dma_start(wsum_t[:], wcb[:, C_WSUM:C_WSUM + CHUNKS * 128])

            # hemisphere-mask dots in [l, n] layout: full fp32 (sign-exact).
            # c' column block already carries TINY - c via the ones row.
            a_all = cpool.tile([128, NC], f32)
            b_all = cpool.tile([128, NC], f32)
            c_all = cpool.tile([128, NC], f32)
            for hf in range(2):
                fs = hf * HF
                for wi, dst in ((0, a_all), (1, b_all), (2, c_all)):
                    pab = ps.tile([128, HF], f32, tag="ph")
                    nc.tensor.matmul(pab[:], wabc_t[:, wi * L:(wi + 1) * L],
                                     nrm4_t[:, fs:fs + HF], start=True, stop=True)
                    nc.vector.tensor_copy(dst[:, fs:fs + HF], pab[:])

            out_ps = opsp.tile([128, NC], f32)
            pending_sum = None

            for C in range(CHUNKS):
                r_m = io.tile([128, NC], f32, tag="rm")
                nc.sync.dma_start(r_m[:], inp[C * 128:(C + 1) * 128, :])
                base = C * BLK_ROWS
                blk0 = io.tile([116, NC], bf16, tag="b0")
                nc.sync.dma_start(blk0[:], inpb[base:base + 116, :])
                blk1 = io.tile([116, NC], bf16, tag="b1")
                nc.sync.dma_start(blk1[:], inpb[base + 116:base + 232, :])
                blk2 = io.tile([36, NC], bf16, tag="b2")
                nc.sync.dma_start(blk2[:], inpb[base + 232:base + 268, :])
                blks = (blk0, blk1, blk2)
                wcst_t = wstage.tile([128, LPC * 128], bf16, tag="wcst")
                nc.sync.dma_start(wcst_t[:],
                                  wcb[:, C * LPC * 128:(C + 1) * LPC * 128])

                # block-layout trig FIRST (gates the PE): ONE in-place Sin per
                # sub-chunk tile; the trailing point rows pass through.
                nc.scalar.activation(blk0[0:112, :], blk0[0:112, :], AF.Sin,
                                     bias=cb_t[0:112, 5:6], scale=PI4)
                nc.scalar.activation(blk1[0:112, :], blk1[0:112, :], AF.Sin,
                                     bias=cb_t[0:112, 5:6], scale=PI4)
                nc.scalar.activation(blk2[0:32, :], blk2[0:32, :], AF.Sin,
                                     bias=cb_t[0:32, 6:7], scale=PI4)

                # s-duplicate this chunk's a/b/c rows into (l,s) layout
                a_C = abcp.tile([128, NC], f32, tag="aC")
                b_C = abcp.tile([128, NC], f32, tag="bC")
                c_C = abcp.tile([128, NC], f32, tag="cC")
                for src, dst in ((a_all, a_C), (b_all, b_C), (c_all, c_C)):
                    dup = src[C * LPC:(C + 1) * LPC, :].unsqueeze(1)
                    dup = dup.broadcast_to((LPC, 8, NC))
                    nc.sync.dma_start(dst[:], dup)

                ct_m = trig.tile([128, NC], f32, tag="ct")
                st_m = trig.tile([128, NC], f32, tag="st")

                for hf in range(2):
                    fs = hf * HF
                    zt = zps.tile([128, HF], f32, tag="zt")
                    # hidden matmuls run LOOK lobes ahead of the z matmuls so
                    # the PE FIFO never parks behind an in-flight relu-drain
                    # (drain latency ~0.9us vs 426ns of PE work per lobe).
                    LOOK = 3
                    hrs = [None] * LPC
                    drain_eng = DRAIN

                    def hidden(j16):
                        k = min(j16 // 7, 2)
                        kv = 116 if k < 2 else 36
                        ph = ps.tile([128, HF], f32, tag="ph")
                        nc.tensor.matmul(ph[:],
                                         wcst_t[0:kv, j16 * 128:(j16 + 1) * 128],
                                         blks[k][0:kv, fs:fs + HF],
                                         start=True, stop=True)
                        hr = hrp.tile([128, HF], bf16, tag="hr")
                        if drain_eng[j16] == "A":
                            nc.scalar.activation(hr[:], ph[:], AF.Relu,
                                                 bias=cb_t[:, 3:4])
                        else:
                            nc.vector.tensor_scalar(hr[:], ph[:], 0.0, 0.0,
                                                    ALU.max, ALU.bypass)
                        hrs[j16] = hr

                    def zmm(j16):
                        j = j16 % 8
                        g = j16 // 8
                        nc.tensor.matmul(zt[64 * g:64 * (g + 1), :],
                                         wsig_t[:, j * 64:(j + 1) * 64],
                                         hrs[j16][:], start=(j == 0),
                                         stop=(j == 7))

                    for j16 in range(LOOK):
                        hidden(j16)
                    for j16 in range(LPC):
                        if j16 + LOOK < LPC:
                            hidden(j16 + LOOK)
                        zmm(j16)
                        if j16 == 4 and pending_sum is not None:
                            pending_sum()
                            pending_sum = None
                    tanhv = work.tile([128, HF], bf16, tag="tanhv")
                    nc.scalar.activation(tanhv[:], zt[:], AF.Tanh,
                                         bias=cb_t[:, 2:3], scale=0.5)
                    if hf == 0:
                        # mask-path trig AFTER the hf0 drains on the ACT FIFO
                        nc.scalar.activation(ct_m[:], r_m[:], AF.Sin,
                                             bias=cb_t[:, 0:1], scale=PI4)
                        nc.scalar.activation(st_m[:], r_m[:], AF.Sin,
                                             bias=cb_t[:, 1:2], scale=PI4)
                    # mask chain (DVE, after this hf's drains in the FIFO)
                    q1 = work.tile([128, HF], f32, tag="q1")
                    q2 = work.tile([128, HF], f32, tag="q2")
                    q3 = work.tile([128, HF], f32, tag="q3")
                    msk = work.tile([128, HF], bf16, tag="msk")
                    nc.vector.scalar_tensor_tensor(q1[:], ct_m[:, fs:fs + HF],
                                                   1.0, a_C[:, fs:fs + HF],
                                                   ALU.mult, ALU.mult)
                    nc.vector.scalar_tensor_tensor(q2[:], st_m[:, fs:fs + HF],
                                                   1.0, b_C[:, fs:fs + HF],
                                                   ALU.mult, ALU.mult)
                    nc.vector.tensor_add(q3[:], q1[:], q2[:])
                    nc.vector.tensor_tensor(msk[:], q3[:], c_C[:, fs:fs + HF],
                                            ALU.is_gt)
                    # vis*msk = 0.5*(tanh+1)*msk; single fused op + ONE matmul
                    tm = work.tile([128, HF], bf16, tag="tm")
                    nc.vector.scalar_tensor_tensor(tm[:], tanhv[:], 1.0, msk[:],
                                                   ALU.add, ALU.mult)

                    def make_sum(C=C, hf=hf, fs=fs, tm=tm):
                        def emit():
                            nc.tensor.matmul(
                                out_ps[:, fs:fs + HF],
                                wsum_t[:, C * 128:(C + 1) * 128], tm[:],
                                start=(C == 0), stop=(C == CHUNKS - 1))
                            if C == CHUNKS - 1:
                                out_sb = cpool.tile([128, HF], f32,
                                                    tag=f"osb{hf}")
                                nc.vector.tensor_copy(out_sb[:],
                                                      out_ps[:, fs:fs + HF])
                                nc.sync.dma_start(out[:, fs:fs + HF],
                                                  out_sb[:])
                        return emit

                    pending_sum = make_sum()

            pending_sum()

    nc.compile()
    return nc


def _host_constants(points, normals, root_rot, lgtSGLobes, lgtSGLambdas,
                    W1, b1, W2, b2):
    f8 = np.float64
    lob = lgtSGLobes.astype(f8)
    l = lob / (np.linalg.norm(lob, axis=-1, keepdims=True) + TINY)
    z = np.zeros_like(l)
    z[:, 2] = 1.0
    U = np.cross(z, l)
    U = U / (np.linalg.norm(U, axis=-1, keepdims=True) + TINY)
    V = np.cross(l, U)
    V = V / (np.linalg.norm(V, axis=-1, keepdims=True) + TINY)
    sharp = lgtSGLambdas[:, 0].astype(f8)
    r_phi = np.minimum(np.arccos(1.0 - 1.0 / sharp), np.pi / 3.0)
    sp, cp = np.sin(r_phi), np.cos(r_phi)

    Wd = root_rot.astype(f8) @ W1[3:].astype(f8)          # [3,H]
    A = sp[:, None] * (U @ Wd)                             # [L,H]
    B = sp[:, None] * (V @ Wd)
    Cc = cp[:, None] * (l @ Wd)
    W1p = W1[:3].astype(f8)                                # [3,H]
    b1f = b1.astype(f8)
    w2 = W2[:, 0].astype(f8)
    w_l = np.exp(sharp * (cp - 1.0))
    scale_l = w_l / (S * w_l + TINY)
    spU = sp[:, None] * U
    spV = sp[:, None] * V
    cpl = cp[:, None] * l

    # wcst: [128, L*128]; col = l*128 + s*16 + h.  Sub-chunk layout {7,7,2}
    # within each 16-lobe chunk; per-lobe rows in its block tile:
    # ct: 8*jj+s -> -A, st: 8*m+8*jj+s -> -B, pc: 16*m..16*m+4 -> W1p,b1-C.
    wcstZ = np.zeros((128, 128, 128), f8)
    wcstV = wcstZ.reshape(128, L, 8, H)
    for ll in range(L):
        pos = ll % LPC
        k = min(pos // 7, 2)
        jj = pos - 7 * k
        m = 7 if k < 2 else 2
        for s in range(8):
            wcstV[8 * jj + s, ll, s, :] = -A[ll]
            wcstV[8 * m + 8 * jj + s, ll, s, :] = -B[ll]
        for d in range(3):
            wcstV[16 * m + d, ll, :, :] = W1p[d]
        wcstV[16 * m + 3, ll, :, :] = (b1f - Cc[ll])[None, :]

    # wabc: [4, 3*L]; per-lobe columns; c' block = -(cpl) with TINY ones-row
    # so the mask compare is q3 > TINY - c.
    wabc = np.zeros((4, 3 * L), f8)
    wabc[0:3, 0:L] = spU.T
    wabc[0:3, L:2 * L] = spV.T
    wabc[0:3, 2 * L:3 * L] = -cpl.T
    wabc[3, 2 * L:3 * L] = TINY

    # wsig: [128, 8*64]; for in-group position p: cols p*64 + l''*8 + s' =
    # w2[h]*delta(s,s')*delta(l'',p)
    wsig = np.zeros((8, H, 8, 8, 8), f8)
    for p in range(8):
        for s in range(8):
            wsig[s, :, p, p, s] = w2
    # wsum: per-chunk [128, L] blocks with HALF the scale (tanh folding);
    # block cc maps chunk-local lobe lp to global output column cc*16+lp.
    wsum = np.zeros((LPC, 8, CHUNKS, L), f8)
    for cc in range(CHUNKS):
        for lp in range(LPC):
            wsum[lp, :, cc, cc * LPC + lp] = 0.5 * scale_l[cc * LPC + lp]

    cbias = np.zeros((128, 8), f8)
    s_of_p = np.arange(128) % 8
    # ACT Sin LUT domain is [-pi, pi]; input is r*pi/4 + bias with r in [0,1),
    # so shift each s-row by a full period where needed to stay in range.
    cos_bias = s_of_p * (np.pi / 4.0) + np.pi / 2.0 - 2.0 * np.pi * (s_of_p >= 2)
    sin_bias = s_of_p * (np.pi / 4.0) - 2.0 * np.pi * (s_of_p >= 4)
    cbias[:, 0] = cos_bias
    cbias[:, 1] = sin_bias
    cbias[:, 2] = float(b2[0]) * 0.5                      # tanh bias = b2/2
    cbias[:, 3] = 0.0                                     # relu bias
    # sub-chunk tile layouts: col5 for m=7 ([ct56|st56]), col6 for m=2
    p = np.arange(128)
    cbias[:, 5] = np.where(p < 56, cos_bias, np.where(p < 112, sin_bias, 0.0))
    cbias[:, 6] = np.where(p < 16, cos_bias, np.where(p < 32, sin_bias, 0.0))

    return dict(wcst=wcstZ.reshape(128, L * 128), wabc=wabc,
                wsig=wsig.reshape(128, 512),
                wsum=wsum.reshape(128, CHUNKS * L), cb=cbias)


def _make_in_maps(inputs):
    import ml_dtypes
    bf16 = np.dtype(ml_dtypes.bfloat16)
    f32 = np.float32

    const = _host_constants(inputs["points"], inputs["normals"],
                            inputs["root_rot"], inputs["lgtSGLobes"],
                            inputs["lgtSGLambdas"], inputs["W1"],
                            inputs["b1"], inputs["W2"], inputs["b2"])

    # wcb: replicated bf16 constant blob
    wcb = np.zeros((128, WCB_COLS), f32)
    wcb[:, C_WCST:C_WCST + L * 128] = const["wcst"]
    wcb[:, C_WSIG:C_WSIG + 512] = const["wsig"]
    wcb[:, C_WSUM:C_WSUM + CHUNKS * 128] = const["wsum"]
    wcb = np.ascontiguousarray(wcb.astype(bf16))

    r_t = np.asarray(inputs["r_theta_random"], f32).transpose(1, 2, 0).reshape(L * S, N)
    pT = np.asarray(inputs["points"], f32).T
    nT = np.asarray(inputs["normals"], f32).T
    ones = np.ones((1, N), f32)

    # inpb: block-layout theta rows (duplicated for cos|sin phases) + pc rows
    pc4 = np.concatenate([pT, ones], axis=0)               # [4, N]
    blocks = []
    for C in range(CHUNKS):
        ch = r_t[C * 128:(C + 1) * 128]                    # [128, N]
        for lo, m in SUBS:
            slab = ch[8 * lo:8 * (lo + m)]                 # [8m, N]
            blocks.append(slab)
            blocks.append(slab)
            blocks.append(pc4)
    inpb = np.concatenate(blocks, axis=0).astype(bf16)     # [2144, N]

    # inp: f32 per-core blob
    in_maps = []
    for c in range(NCORES):
        sl = slice(c * NC, (c + 1) * NC)
        inp = np.zeros((INP_ROWS, NC), f32)
        inp[R_RT:R_RT + L * S] = r_t[:, sl]
        inp[R_NRM:R_NRM + 3] = nT[:, sl]
        inp[R_NRM + 3] = 1.0
        inp[R_WABC:R_WABC + 4, 0:3 * L] = const["wabc"]
        inp[R_CB:R_CB + 128, 0:8] = const["cb"]
        in_maps.append({
            "inp": np.ascontiguousarray(inp),
            "inpb": np.ascontiguousarray(inpb[:, sl]),
            "wcb": wcb,
        })
    return in_maps


def kernel(points, normals, root_rot, lgtSGLobes, lgtSGLambdas,
           r_theta_random, W1, b1, W2, b2):
    global _PROG
    from concourse.bass_utils import run_bass_kernel_spmd

    if _PROG is None:
        _PROG = _build_program()
    nc = _PROG

    in_maps = _make_in_maps(dict(
        points=points, normals=normals, root_rot=root_rot,
        lgtSGLobes=lgtSGLobes, lgtSGLambdas=lgtSGLambdas,
        r_theta_random=r_theta_random, W1=W1, b1=b1, W2=W2, b2=b2))

    res = run_bass_kernel_spmd(nc, in_maps, list(range(NCORES)))

    f32 = np.float32
    out_full = np.empty((N, L), f32)
    for c in range(NCORES):
        out_full[c * NC:(c + 1) * NC, :] = res.results[c]["out"].T
    return out_full
